# revision 1
# baseline (speedup 1.0000x reference)
"""Cantor global attention kernel for Trainium2 (8 NeuronCores, SPMD).

Strategy: data-parallel over batch B=64 -> 8 cores x 8 rows each.
Per core, every expert slab [8, 4096] is flattened to SBUF [128, 256]
(partition = b*16 + p//256, col = p%256); experts sit side by side in
the free dimension, grouped 4 per tile.  The W=3 neighbor gather and
the beta/temperature gating are folded into per-(e,w) instruction
operand offsets and exp-activation scale immediates, baked at build
time from the runtime routes/betas/temperature values (tiny [16,3]
control-plane tensors).

Engine placement (per core, f32).  DVE 2-operand ops and GpSimd ops
serialize on the shared SBUF port pair, so GpSimd runs NO tensor ops -
only SWDGE descriptor generation:
  - projection averaging (all of Q,K,V): DMA-accumulate (CCE add in
    the SDMA engines) - zero compute-engine cost
  - t_w = Qs*Ks:      DVE tensor_mul, run-batched: route slots are
    permuted per expert (softmax over w is slot-invariant) so the
    route offset j-e is locally constant and one instruction covers
    several experts
  - e_w = exp(c*t):   ScalarE activation, scale=c_ew immediate, in-place
  - prod_w = e_w*Vs:  DVE, run-batched like t
  - den|num = sum_w:  DVE adds over a combined [e3|p3] layout - one add
    pass produces both reductions
  - r = 0.5/den = exp(-ln(den)+ln(.5)): ScalarE, func-clustered per
    group pair to limit ACT table reloads
  - out = num*r:      DVE mul
"""

import math

import numpy as np

import concourse.bass as bass
import concourse.mybir as mybir
from concourse import bacc, tile
from concourse.bass_utils import run_bass_kernel_spmd

E, NPROJ, B, P = 16, 2, 64, 4096
W = 3
EXPERT_DIM = 128
NCORES = 8
BS = B // NCORES          # 8 batch rows per core
COLS = 256                # free-dim columns per expert slab
PH = P // COLS            # 16 partition sub-blocks per batch row
PART = BS * PH            # 128 SBUF partitions
GROUP = 4                 # experts per tile group
NG = E // GROUP           # 4 groups
GC = GROUP * COLS         # 1024 cols per group tile
WGC = W * GC              # e3 / p3 section size in the combined tile

F32 = mybir.dt.float32
EXPF = mybir.ActivationFunctionType.Exp
LNF = mybir.ActivationFunctionType.Ln
ADD = mybir.AluOpType.add


def _runs(pairs):
    """Split [(le, j), ...] into maximal runs with consecutive le and j
    within one j-group."""
    runs = []
    for le, j in pairs:
        if (runs and runs[-1][0] + runs[-1][2] == le
                and runs[-1][1] + runs[-1][2] == j
                and (runs[-1][1] // GROUP == j // GROUP)):
            runs[-1][2] += 1
        else:
            runs.append([le, j, 1])
    return runs


def _build_nc(routes: np.ndarray, coef: np.ndarray):
    nc = bacc.Bacc("TRN2", target_bir_lowering=False, debug=False,
                   num_devices=NCORES)

    q_d = nc.dram_tensor("q", [E, NPROJ, BS, P], F32, kind="ExternalInput")
    k_d = nc.dram_tensor("k", [E, NPROJ, BS, P], F32, kind="ExternalInput")
    v_d = nc.dram_tensor("v", [E, NPROJ, BS, P], F32, kind="ExternalInput")
    o_d = nc.dram_tensor("out", [BS, E * P], F32, kind="ExternalOutput")

    # DRAM views: [(b ph), e, n, c]
    def lview(t):
        return t.ap().rearrange("e n b (ph c) -> (b ph) e n c", c=COLS)

    qv, kv, vv = lview(q_d), lview(k_d), lview(v_d)
    ov = o_d.ap().rearrange("b (e ph c) -> b ph e c", ph=PH, c=COLS)

    # group g of experts is ready once groups up to ready_g[g] are loaded
    ready_g = [max(g, int(routes[g * GROUP:(g + 1) * GROUP].max()) // GROUP)
               for g in range(NG)]

    with tile.TileContext(nc) as tc:
        with (
            tc.tile_pool(name="raw", bufs=4) as raw_p,
            tc.tile_pool(name="qs", bufs=NG) as qs_p,
            tc.tile_pool(name="ks", bufs=NG) as ks_p,
            tc.tile_pool(name="vs", bufs=NG) as vs_p,
            tc.tile_pool(name="tp", bufs=3) as tp_p,
            tc.tile_pool(name="dn", bufs=2) as dn_p,
            tc.tile_pool(name="sm", bufs=2) as sm_p,
        ):
            qs, ks, vs = [], [], []

            def emit_phase1(g):
                """t, exp, prod for expert group g into a combined tile
                tp = [w0e3|w1e3|w2e3 | w0p3|w1p3|w2p3]."""
                e0 = g * GROUP
                tp = tp_p.tile([PART, 2 * WGC], F32, name="tp", tag="tp")
                for w in range(W):
                    pairs = [(le, int(routes[e0 + le, w]))
                             for le in range(GROUP)]
                    for le, j, L in _runs(pairs):
                        gj, lj = j // GROUP, j % GROUP
                        nc.vector.tensor_mul(
                            tp[:, w * GC + le * COLS:
                               w * GC + (le + L) * COLS],
                            qs[g][:, le * COLS:(le + L) * COLS],
                            ks[gj][:, lj * COLS:(lj + L) * COLS])
                    for le in range(GROUP):
                        sl = slice(w * GC + le * COLS,
                                   w * GC + (le + 1) * COLS)
                        nc.scalar.activation(tp[:, sl], tp[:, sl], EXPF,
                                             bias=0.0,
                                             scale=float(coef[e0 + le, w]))
                for w in range(W):
                    pairs = [(le, int(routes[e0 + le, w]))
                             for le in range(GROUP)]
                    for le, j, L in _runs(pairs):
                        gj, lj = j // GROUP, j % GROUP
                        nc.vector.tensor_mul(
                            tp[:, WGC + w * GC + le * COLS:
                               WGC + w * GC + (le + L) * COLS],
                            tp[:, w * GC + le * COLS:
                               w * GC + (le + L) * COLS],
                            vs[gj][:, lj * COLS:(lj + L) * COLS])
                # dn = [den | num], both w-sums in one add pass; frees tp
                dn = dn_p.tile([PART, 2 * GC], F32, name="dn", tag="dn")
                iv = [tp[:].rearrange("p (k w c) -> p k w c", k=2, w=W)
                      [:, :, w, :] for w in range(W)]
                dnv = dn[:].rearrange("p (k c) -> p k c", k=2)
                nc.vector.tensor_add(dnv, iv[0], iv[1])
                nc.vector.tensor_add(dnv, dnv, iv[2])
                return dn

            def emit_finale(g, dn):
                """recip / out / stores for one group (all DVE + stores).
                Halved so the first stores issue while the second half of
                the normalize still runs - shortens the kernel tail."""
                og = sm_p.tile([PART, GC], F32, name="og", tag="og")
                for h in range(2):
                    hc = slice(h * GC // 2, (h + 1) * GC // 2)
                    rcp = sm_p.tile([PART, GC // 2], F32, name="rcp",
                                    tag="rcp", bufs=4)
                    nc.vector.reciprocal_approx_fast(rcp[:], dn[:, hc])
                    nc.vector.scalar_tensor_tensor(
                        og[:, hc], dn[:, GC + h * GC // 2:
                                      GC + (h + 1) * GC // 2], 0.5, rcp[:],
                        mybir.AluOpType.mult, mybir.AluOpType.mult)
                    for le in range(h * GROUP // 2, (h + 1) * GROUP // 2):
                        nc.sync.dma_start(ov[:, :, g * GROUP + le],
                                          og[:, le * COLS:(le + 1) * COLS])

            qs, ks, vs = [None] * NG, [None] * NG, [None] * NG
            # wave-gating: non-critical load DMAs wait for the loads that
            # unblock the first compute groups, so the SDMA engines aren't
            # round-robining against them on the critical path.
            gate_insts = []
            rings = [nc.sync, nc.scalar]
            ring_i = [0]

            def ring():
                ring_i[0] += 1
                return rings[ring_i[0] % 2]

            def gate(inst):
                for gi in gate_insts:
                    tile.add_dep_helper(inst.ins, gi.ins, sync=True,
                                        reason="load wave gating")

            def load_plain(dview, sums, s_p, g, wave1):
                """Plain load of both projections + DVE averaging."""
                es = slice(g * GROUP, (g + 1) * GROUP)
                s = s_p.tile([PART, GC], F32, name="s", tag="s")
                sv = s[:].rearrange("p (e c) -> p e c", e=GROUP)
                raw = raw_p.tile([PART, NPROJ * GC], F32, name="raw",
                                 tag="raw")
                rv = raw[:].rearrange("p (e n c) -> p e n c",
                                      e=GROUP, n=NPROJ)
                nh = GROUP // 2 if wave1 else GROUP
                for h0 in range(0, GROUP, nh):
                    hs = slice(g * GROUP + h0, g * GROUP + h0 + nh)
                    i0 = ring().dma_start(rv[:, h0:h0 + nh], dview[:, hs])
                    if wave1:
                        gate_insts.append(i0)
                    else:
                        gate(i0)
                    nc.vector.tensor_add(sv[:, h0:h0 + nh],
                                         rv[:, h0:h0 + nh, 0],
                                         rv[:, h0:h0 + nh, 1])
                sums[g] = s

            def load_accum(dview, sums, s_p, g):
                """Plain proj0 + DMA-accumulate proj1 (latency hides)."""
                es = slice(g * GROUP, (g + 1) * GROUP)
                s = s_p.tile([PART, GC], F32, name="s", tag="s")
                gate(ring().dma_start(s[:], dview[:, es, 0]))
                nc.gpsimd.dma_start(s[:], dview[:, es, 1], accum_op=ADD)
                sums[g] = s

            def load(dview, sums, s_p, g, wave1, is_v=False):
                if g >= 2:
                    load_accum(dview, sums, s_p, g)
                else:
                    load_plain(dview, sums, s_p, g, wave1)

            # Need-driven load order: emit exactly what unblocks the next
            # group's compute; late groups use DMA-accumulate.
            order = sorted(range(NG), key=lambda g: (ready_g[g], g))
            first = order[0]
            for g in order:
                wave1 = g == first
                rg = routes[g * GROUP:(g + 1) * GROUP]
                kneed = sorted({int(j) // GROUP for j in rg.flatten()})
                if qs[g] is None:
                    load(qv, qs, qs_p, g, wave1)
                for gj in kneed:
                    if ks[gj] is None:
                        load(kv, ks, ks_p, gj, wave1)
                for gj in kneed:
                    if vs[gj] is None:
                        load(vv, vs, vs_p, gj, False, is_v=True)
                dn = emit_phase1(g)
                emit_finale(g, dn)

    nc.compile()
    return nc


_cache: dict = {}


def _get_nc(routes: np.ndarray, coef: np.ndarray):
    key = (routes.tobytes(), coef.tobytes())
    if key not in _cache:
        _cache[key] = _build_nc(routes, coef)
    return _cache[key]


def kernel(Q_proj, K_proj, V_proj, betas, temperature, routes, num_patches):
    Q = np.asarray(Q_proj, dtype=np.float32)
    K = np.asarray(K_proj, dtype=np.float32)
    V = np.asarray(V_proj, dtype=np.float32)
    betas = np.asarray(betas, dtype=np.float32)
    temp = np.asarray(temperature, dtype=np.float32)
    routes = np.asarray(routes, dtype=np.int32)

    # Host control-plane: beta gating + scale folded into one coefficient
    # per (expert, neighbor).  0.25 = the two projection means of Q and K
    # (sums are averaged); V's 0.5 is folded into the reciprocal's bias.
    scale = np.float32(np.sqrt(np.float32(EXPERT_DIM))) * np.abs(temp[0])
    gate = np.where(routes != np.arange(E, dtype=np.int32)[:, None],
                    np.float32(1.0) / (np.float32(1.0) + np.exp(-betas)),
                    np.float32(1.0)).astype(np.float32)
    coef = (np.float32(0.25) * gate / scale).astype(np.float32)

    # Permute each expert's route slots so the offset j-e is sorted:
    # softmax over w is slot-invariant, and locally-constant offsets let
    # the builder batch consecutive experts into single instructions.
    order = np.argsort(routes - np.arange(E, dtype=np.int32)[:, None],
                       axis=1, kind="stable")
    routes_p = np.take_along_axis(routes, order, axis=1)
    coef_p = np.take_along_axis(coef, order, axis=1)

    nc = _get_nc(routes_p, coef_p)
    in_maps = [
        {
            "q": np.ascontiguousarray(Q[:, :, c * BS:(c + 1) * BS, :]),
            "k": np.ascontiguousarray(K[:, :, c * BS:(c + 1) * BS, :]),
            "v": np.ascontiguousarray(V[:, :, c * BS:(c + 1) * BS, :]),
        }
        for c in range(NCORES)
    ]
    res = run_bass_kernel_spmd(nc, in_maps, list(range(NCORES)))
    return np.concatenate([res.results[c]["out"] for c in range(NCORES)],
                          axis=0)



# revision 3
# speedup vs baseline: 1.3087x; 1.3087x over previous
"""Cantor global attention kernel for Trainium2 (8 NeuronCores, SPMD).

Strategy: data-parallel over batch B=64 -> 8 cores x 8 rows each.
All device tensors are 16-bit: Q/K (and the pre-exp score t) in fp16
for exponent accuracy, everything after the exp in bf16 for range
safety (scores reach ~|20| so e^t needs bf16's exponent range).  The
host uploads inputs already transposed into the SBUF layout
[proj][128 part][e*256 col] so every DMA is a few large contiguous
descriptors, and converts the bf16 output back to f32.

Per core, partition = b*16 + p//256; each expert owns 256 columns;
the W=3 neighbor gather becomes column offsets baked from the runtime
routes (slot-permuted so slot0 = self).

Engine placement (per core):
  - projection averaging (Q,K,V): DMA-accumulate (CCE add) - free
  - t_w = Qs*Ks_route:  DVE tensor_mul fp16 (2x packed), run-batched
  - gate: t *= sigmoid(beta) per non-self (e,w): DVE tensor_scalar
    (4x mode), immediate baked from betas
  - e_w = exp(esc*t):   ScalarE activation, uniform scale immediate
    esc = 0.25/(sqrt(128)*|temp|)  ->  fully batched big instructions
  - prod_w = e_w*Vs:    DVE tensor_mul bf16, run-batched
  - den|num = sum_w:    2 DVE adds per group over a [k=2,w=3] strided
    view covering both reductions
  - r = 0.5/den:        ScalarE ln (fp32 out) then exp(-x+ln 0.5)
  - out = num*r:        DVE mul, stored bf16
"""

import math

import numpy as np

import concourse.bass as bass
import concourse.mybir as mybir
from concourse import bacc, tile
from concourse.bass_utils import run_bass_kernel_spmd

E, NPROJ, B, P = 16, 2, 64, 4096
W = 3
EXPERT_DIM = 128
NCORES = 8
BS = B // NCORES          # 8 batch rows per core
COLS = 256                # free-dim columns per expert slab
PH = P // COLS            # 16 partition sub-blocks per batch row
PART = BS * PH            # 128 SBUF partitions
EC = E * COLS             # 4096 cols per w-block
GROUP = 4                 # experts per compute group
NG = E // GROUP           # 4 groups
GC = GROUP * COLS         # 1024 cols per group
QW = 1024                 # load quarter width (cols)

F16 = mybir.dt.float16
BF16 = mybir.dt.bfloat16
F32 = mybir.dt.float32
EXPF = mybir.ActivationFunctionType.Exp
LNF = mybir.ActivationFunctionType.Ln
MULT = mybir.AluOpType.mult
ADD = mybir.AluOpType.add


def _runs(pairs):
    """Split [(e, j), ...] into maximal runs of consecutive e and j."""
    runs = []
    for e, j in pairs:
        if runs and runs[-1][0] + runs[-1][2] == e and runs[-1][1] + runs[-1][2] == j:
            runs[-1][2] += 1
        else:
            runs.append([e, j, 1])
    return runs


def _build_nc(routes_s: np.ndarray, gates_s: np.ndarray, esc: float):
    nc = bacc.Bacc("TRN2", target_bir_lowering=False, debug=False,
                   num_devices=NCORES)

    q_d = nc.dram_tensor("q", [NPROJ, PART, EC], F16, kind="ExternalInput")
    k_d = nc.dram_tensor("k", [NPROJ, PART, EC], F16, kind="ExternalInput")
    v_d = nc.dram_tensor("v", [NPROJ, PART, EC], F16, kind="ExternalInput")
    o_d = nc.dram_tensor("out", [PART, EC], BF16, kind="ExternalOutput")

    # per-group, per-slot runs split at group boundaries
    runs_g = [[] for _ in range(NG)]
    for g in range(NG):
        for w in range(W):
            pairs = [(e, int(routes_s[e, w]))
                     for e in range(g * GROUP, (g + 1) * GROUP)]
            for e0, j0, L in _runs(pairs):
                runs_g[g].append((w, e0, j0, L))

    with tile.TileContext(nc) as tc:
        with (
            tc.tile_pool(name="io", bufs=1) as io_p,
            tc.tile_pool(name="mid", bufs=1) as mid_p,
        ):
            qs = io_p.tile([PART, EC], F16, name="qs", tag="qs")
            ks = io_p.tile([PART, EC], F16, name="ks", tag="ks")
            vs = io_p.tile([PART, EC], F16, name="vs", tag="vs")
            tp = mid_p.tile([PART, W * EC], F16, name="tp", tag="tp")
            epr = mid_p.tile([PART, 2 * W * EC], BF16, name="epr", tag="epr")
            dn = mid_p.tile([PART, 2 * EC], BF16, name="dn", tag="dn")
            lnt = mid_p.tile([PART, EC], F32, name="lnt", tag="lnt")
            rr = mid_p.tile([PART, EC], BF16, name="rr", tag="rr")
            og = mid_p.tile([PART, EC], BF16, name="og", tag="og")

            qv, kv, vv = q_d.ap(), k_d.ap(), v_d.ap()
            ov = o_d.ap()

            def load(dst, src, quarter):
                c0, c1 = quarter * QW, (quarter + 1) * QW
                nc.sync.dma_start(dst[:, c0:c1], src[0][:, c0:c1])
                nc.gpsimd.dma_start(dst[:, c0:c1], src[1][:, c0:c1],
                                    accum_op=ADD)

            # strided views
            tpv = tp[:].rearrange("p (w c) -> p w c", w=W)
            epv = epr[:].rearrange("p (k w c) -> p k w c", k=2, w=W)
            dnv = dn[:].rearrange("p (k c) -> p k c", k=2)

            def compute(g):
                c0, c1 = g * GC, (g + 1) * GC
                # t = Qs * Ks[route]
                for w, e0, j0, L in runs_g[g]:
                    nc.vector.tensor_mul(
                        tp[:, w * EC + e0 * COLS: w * EC + (e0 + L) * COLS],
                        qs[:, e0 * COLS:(e0 + L) * COLS],
                        ks[:, j0 * COLS:(j0 + L) * COLS])
                # gate the non-self slots (slot0 is self, gate 1)
                for w in range(1, W):
                    for e in range(g * GROUP, (g + 1) * GROUP):
                        sl = slice(w * EC + e * COLS, w * EC + (e + 1) * COLS)
                        nc.vector.tensor_scalar_mul(
                            tp[:, sl], tp[:, sl], float(gates_s[e, w]))
                # e = exp(esc * t), one instruction per group over all 3 slots
                nc.scalar.activation(epv[:, 0, :, c0:c1], tpv[:, :, c0:c1],
                                     EXPF, bias=0.0, scale=esc)
                # prod = e * Vs[route]
                for w, e0, j0, L in runs_g[g]:
                    nc.vector.tensor_mul(
                        epr[:, (W + w) * EC + e0 * COLS:
                            (W + w) * EC + (e0 + L) * COLS],
                        epr[:, w * EC + e0 * COLS: w * EC + (e0 + L) * COLS],
                        vs[:, j0 * COLS:(j0 + L) * COLS])
                # den | num sums over w in two adds
                nc.vector.tensor_add(dnv[:, :, c0:c1], epv[:, :, 0, c0:c1],
                                     epv[:, :, 1, c0:c1])
                nc.vector.tensor_add(dnv[:, :, c0:c1], dnv[:, :, c0:c1],
                                     epv[:, :, 2, c0:c1])
                # r = 0.5/den = exp(-ln(2*den)); the 0.5 rides the ln scale
                nc.scalar.activation(lnt[:, c0:c1], dn[:, c0:c1], LNF,
                                     bias=0.0, scale=2.0)
                nc.scalar.activation(rr[:, c0:c1], lnt[:, c0:c1], EXPF,
                                     bias=0.0, scale=-1.0)
                # out = num * r
                nc.vector.tensor_mul(og[:, c0:c1], dn[:, EC + c0:EC + c1],
                                     rr[:, c0:c1])
                nc.scalar.dma_start(ov[:, c0:c1], og[:, c0:c1])

            # need-driven load + compute interleave; group order g3 first
            # (its routes stay inside quarters 2-3).
            load(ks, kv, 2)
            load(ks, kv, 3)
            load(qs, qv, 3)
            load(vs, vv, 2)
            load(vs, vv, 3)
            compute(3)
            load(ks, kv, 0)
            load(ks, kv, 1)
            load(qs, qv, 0)
            load(vs, vv, 0)
            load(vs, vv, 1)
            compute(0)
            load(qs, qv, 1)
            compute(1)
            load(qs, qv, 2)
            compute(2)

    nc.compile()
    return nc


_cache: dict = {}


def _get_nc(routes_s: np.ndarray, gates_s: np.ndarray, esc: float):
    key = (routes_s.tobytes(), gates_s.tobytes(), float(esc))
    if key not in _cache:
        _cache[key] = _build_nc(routes_s, gates_s, esc)
    return _cache[key]


def _slot_sort(routes: np.ndarray, betas: np.ndarray):
    """Slot-permute so slot0 = self (gate 1); others sorted by offset."""
    gate = np.where(routes != np.arange(E, dtype=np.int32)[:, None],
                    1.0 / (1.0 + np.exp(-betas.astype(np.float64))),
                    1.0)
    routes_s = np.zeros((E, W), np.int32)
    gates_s = np.ones((E, W), np.float64)
    for e in range(E):
        slots = list(range(W))
        self_w = [w for w in slots if routes[e, w] == e]
        assert self_w, f"expert {e} missing self route"
        rest = [w for w in slots if w != self_w[0]]
        rest.sort(key=lambda w: int(routes[e, w]) - e)
        order = [self_w[0]] + rest
        routes_s[e] = routes[e, order]
        gates_s[e] = gate[e, order]
    return routes_s, gates_s.astype(np.float32)


def kernel(Q_proj, K_proj, V_proj, betas, temperature, routes, num_patches):
    Q = np.asarray(Q_proj, dtype=np.float32)
    K = np.asarray(K_proj, dtype=np.float32)
    V = np.asarray(V_proj, dtype=np.float32)
    betas = np.asarray(betas, dtype=np.float32)
    temp = np.asarray(temperature, dtype=np.float32)
    routes = np.asarray(routes, dtype=np.int32)
    assert int(num_patches) == E * P

    # Qs = Q0+Q1 (2x the mean); the 0.25 from both means is folded into
    # the exp scale esc together with sqrt(d)*|temperature|.
    esc = float(0.25 / (np.sqrt(np.float32(EXPERT_DIM)) * np.abs(temp[0])))
    routes_s, gates_s = _slot_sort(routes, betas)
    nc = _get_nc(routes_s, gates_s, esc)

    def prep(X):
        # [E, NPROJ, BS, P] -> [NPROJ, (b ph), (e c)] fp16
        return np.ascontiguousarray(
            X.reshape(E, NPROJ, BS, PH, COLS).transpose(1, 2, 3, 0, 4)
            .reshape(NPROJ, PART, EC).astype(np.float16))

    in_maps = []
    for c in range(NCORES):
        sl = slice(c * BS, (c + 1) * BS)
        in_maps.append({
            "q": prep(Q[:, :, sl, :]),
            "k": prep(K[:, :, sl, :]),
            "v": prep(V[:, :, sl, :]),
        })

    res = run_bass_kernel_spmd(nc, in_maps, list(range(NCORES)))
    out = np.empty((B, E * P), np.float32)
    for c in range(NCORES):
        o = np.asarray(res.results[c]["out"]).astype(np.float32)
        out[c * BS:(c + 1) * BS] = (
            o.reshape(BS, PH, E, COLS).transpose(0, 2, 1, 3)
            .reshape(BS, E * P))
    return out


# revision 6
# speedup vs baseline: 1.4851x; 1.1348x over previous
"""Cantor global attention kernel for Trainium2 (8 NeuronCores, SPMD).

Strategy: data-parallel over batch B=64 -> 8 cores x 8 rows each.
All device tensors are 16-bit: Q/K (and the pre-exp score t) in fp16
for exponent accuracy, everything after the exp in bf16 for range
safety (scores reach ~|20| so e^t needs bf16's exponent range).  The
host uploads inputs already transposed into the SBUF layout
[proj][128 part][e*256 col] so every DMA is a few large contiguous
descriptors, and converts the bf16 output back to f32.

Per core, partition = b*16 + p//256; each expert owns 256 columns;
the W=3 neighbor gather becomes column offsets baked from the runtime
routes (slot-permuted so slot0 = self).

Engine placement (per core):
  - projection averaging (Q,K,V): DMA-accumulate (CCE add) - free
  - t_w = Qs*Ks_route:  DVE tensor_mul fp16 (2x packed), run-batched
  - gate: t *= sigmoid(beta) per non-self (e,w): DVE tensor_scalar
    (4x mode), immediate baked from betas
  - e_w = exp(esc*t):   ScalarE activation, uniform scale immediate
    esc = 0.25/(sqrt(128)*|temp|)  ->  fully batched big instructions
  - prod_w = e_w*Vs:    DVE tensor_mul bf16, run-batched
  - den|num = sum_w:    2 DVE adds per group over a [k=2,w=3] strided
    view covering both reductions
  - r = 0.5/den:        ScalarE ln (fp32 out) then exp(-x+ln 0.5)
  - out = num*r:        DVE mul, stored bf16
"""

import math

import numpy as np

import concourse.bass as bass
import concourse.mybir as mybir
from concourse import bacc, tile
from concourse.bass_utils import run_bass_kernel_spmd

E, NPROJ, B, P = 16, 2, 64, 4096
W = 3
EXPERT_DIM = 128
NCORES = 8
BS = B // NCORES          # 8 batch rows per core
COLS = 256                # free-dim columns per expert slab
PH = P // COLS            # 16 partition sub-blocks per batch row
PART = BS * PH            # 128 SBUF partitions
EC = E * COLS             # 4096 cols per w-block
GROUP = 4                 # experts per compute group
NG = E // GROUP           # 4 groups
GC = GROUP * COLS         # 1024 cols per group
ACT_SET_LN_EXP = 6        # act_info.json natural_log_exp_and_others

F16 = mybir.dt.float16
BF16 = mybir.dt.bfloat16
F32 = mybir.dt.float32
EXPF = mybir.ActivationFunctionType.Exp
LNF = mybir.ActivationFunctionType.Ln
MULT = mybir.AluOpType.mult
ADD = mybir.AluOpType.add


def _runs(pairs):
    """Split [(e, j), ...] into maximal runs of consecutive e and j."""
    runs = []
    for e, j in pairs:
        if runs and runs[-1][0] + runs[-1][2] == e and runs[-1][1] + runs[-1][2] == j:
            runs[-1][2] += 1
        else:
            runs.append([e, j, 1])
    return runs


def _build_nc(routes_s: np.ndarray, gates_s: np.ndarray, esc: float):
    nc = bacc.Bacc("TRN2", target_bir_lowering=False, debug=False,
                   num_devices=NCORES)

    q_d = nc.dram_tensor("q", [NPROJ, PART, EC], F16, kind="ExternalInput")
    k_d = nc.dram_tensor("k", [NPROJ, PART, EC], F16, kind="ExternalInput")
    v_d = nc.dram_tensor("v", [NPROJ, PART, EC], F16, kind="ExternalInput")
    o_d = nc.dram_tensor("out", [PART, EC], BF16, kind="ExternalOutput")

    # per-group, per-slot runs split at group boundaries
    runs_g = [[] for _ in range(NG)]
    for g in range(NG):
        for w in range(W):
            pairs = [(e, int(routes_s[e, w]))
                     for e in range(g * GROUP, (g + 1) * GROUP)]
            for e0, j0, L in _runs(pairs):
                runs_g[g].append((w, e0, j0, L))

    with tile.TileContext(nc) as tc:
        with (
            tc.tile_pool(name="io", bufs=1) as io_p,
            tc.tile_pool(name="mid", bufs=1) as mid_p,
        ):
            qs = io_p.tile([PART, EC], F16, name="qs", tag="qs")
            ks = io_p.tile([PART, EC], F16, name="ks", tag="ks")
            vs = io_p.tile([PART, EC], F16, name="vs", tag="vs")
            tp = mid_p.tile([PART, W * EC], F16, name="tp", tag="tp")
            epr = mid_p.tile([PART, 2 * W * EC], BF16, name="epr", tag="epr")
            dn = mid_p.tile([PART, 2 * EC], BF16, name="dn", tag="dn")
            lnt = mid_p.tile([PART, EC], F32, name="lnt", tag="lnt")
            rr = mid_p.tile([PART, EC], BF16, name="rr", tag="rr")
            og = mid_p.tile([PART, EC], BF16, name="og", tag="og")

            qv, kv, vv = q_d.ap(), k_d.ap(), v_d.ap()
            ov = o_d.ap()

            # Pin the ACT table set that holds BOTH exp and ln so the
            # compiler's table-load pass never needs to switch sets.
            nc.scalar.add_instruction(mybir.InstLoadActFuncSet(
                name=nc.get_next_instruction_name(),
                act_func_set_id=ACT_SET_LN_EXP, ins=[], outs=[]))

            def load(dst, src, half, ring):
                c0, c1 = half * (EC // 2), (half + 1) * (EC // 2)
                ring.dma_start(dst[:, c0:c1], src[0][:, c0:c1])
                nc.gpsimd.dma_start(dst[:, c0:c1], src[1][:, c0:c1],
                                    accum_op=ADD)

            # strided views
            tpv = tp[:].rearrange("p (w c) -> p w c", w=W)
            epv = epr[:].rearrange("p (k w c) -> p k w c", k=2, w=W)
            dnv = dn[:].rearrange("p (k c) -> p k c", k=2)

            def compute(g):
                c0, c1 = g * GC, (g + 1) * GC
                # t = Qs * Ks[route]
                for w, e0, j0, L in runs_g[g]:
                    nc.vector.tensor_mul(
                        tp[:, w * EC + e0 * COLS: w * EC + (e0 + L) * COLS],
                        qs[:, e0 * COLS:(e0 + L) * COLS],
                        ks[:, j0 * COLS:(j0 + L) * COLS])
                # gate the non-self slots (slot0 is self, gate 1)
                for w in range(1, W):
                    for e in range(g * GROUP, (g + 1) * GROUP):
                        sl = slice(w * EC + e * COLS, w * EC + (e + 1) * COLS)
                        nc.vector.tensor_scalar_mul(
                            tp[:, sl], tp[:, sl], float(gates_s[e, w]))
                # e = exp(esc * t), one instruction per group over all 3 slots
                nc.scalar.activation(epv[:, 0, :, c0:c1], tpv[:, :, c0:c1],
                                     EXPF, bias=0.0, scale=esc)
                # prod = e * Vs[route]
                for w, e0, j0, L in runs_g[g]:
                    nc.vector.tensor_mul(
                        epr[:, (W + w) * EC + e0 * COLS:
                            (W + w) * EC + (e0 + L) * COLS],
                        epr[:, w * EC + e0 * COLS: w * EC + (e0 + L) * COLS],
                        vs[:, j0 * COLS:(j0 + L) * COLS])
                # den | num sums over w in two adds
                nc.vector.tensor_add(dnv[:, :, c0:c1], epv[:, :, 0, c0:c1],
                                     epv[:, :, 1, c0:c1])
                nc.vector.tensor_add(dnv[:, :, c0:c1], dnv[:, :, c0:c1],
                                     epv[:, :, 2, c0:c1])
                # r = 0.5/den = exp(-ln(2*den)); the 0.5 rides the ln scale
                nc.scalar.activation(lnt[:, c0:c1], dn[:, c0:c1], LNF,
                                     bias=0.0, scale=2.0)
                nc.scalar.activation(rr[:, c0:c1], lnt[:, c0:c1], EXPF,
                                     bias=0.0, scale=-1.0)
                # out = num * r
                nc.vector.tensor_mul(og[:, c0:c1], dn[:, EC + c0:EC + c1],
                                     rr[:, c0:c1])

            def store(half):
                c0, c1 = half * (EC // 2), (half + 1) * (EC // 2)
                nc.scalar.dma_start(ov[:, c0:c1], og[:, c0:c1])

            # need-driven load + compute interleave; groups 3,2 only touch
            # k/v columns that live in the halves loaded first.
            load(ks, kv, 1, nc.sync)
            load(qs, qv, 1, nc.scalar)
            load(vs, vv, 1, nc.sync)
            compute(3)
            load(ks, kv, 0, nc.sync)
            load(qs, qv, 0, nc.scalar)
            load(vs, vv, 0, nc.sync)
            compute(2)
            store(1)
            compute(0)
            compute(1)
            store(0)

    nc.compile()
    return nc


_cache: dict = {}


def _get_nc(routes_s: np.ndarray, gates_s: np.ndarray, esc: float):
    key = (routes_s.tobytes(), gates_s.tobytes(), float(esc))
    if key not in _cache:
        _cache[key] = _build_nc(routes_s, gates_s, esc)
    return _cache[key]


def _slot_sort(routes: np.ndarray, betas: np.ndarray):
    """Slot-permute so slot0 = self (gate 1); others sorted by offset."""
    gate = np.where(routes != np.arange(E, dtype=np.int32)[:, None],
                    1.0 / (1.0 + np.exp(-betas.astype(np.float64))),
                    1.0)
    routes_s = np.zeros((E, W), np.int32)
    gates_s = np.ones((E, W), np.float64)
    for e in range(E):
        slots = list(range(W))
        self_w = [w for w in slots if routes[e, w] == e]
        assert self_w, f"expert {e} missing self route"
        rest = [w for w in slots if w != self_w[0]]
        rest.sort(key=lambda w: int(routes[e, w]) - e)
        order = [self_w[0]] + rest
        routes_s[e] = routes[e, order]
        gates_s[e] = gate[e, order]
    return routes_s, gates_s.astype(np.float32)


def kernel(Q_proj, K_proj, V_proj, betas, temperature, routes, num_patches):
    Q = np.asarray(Q_proj, dtype=np.float32)
    K = np.asarray(K_proj, dtype=np.float32)
    V = np.asarray(V_proj, dtype=np.float32)
    betas = np.asarray(betas, dtype=np.float32)
    temp = np.asarray(temperature, dtype=np.float32)
    routes = np.asarray(routes, dtype=np.int32)
    assert int(num_patches) == E * P

    # Qs = Q0+Q1 (2x the mean); the 0.25 from both means is folded into
    # the exp scale esc together with sqrt(d)*|temperature|.
    esc = float(0.25 / (np.sqrt(np.float32(EXPERT_DIM)) * np.abs(temp[0])))
    routes_s, gates_s = _slot_sort(routes, betas)
    nc = _get_nc(routes_s, gates_s, esc)

    def prep(X):
        # [E, NPROJ, BS, P] -> [NPROJ, (b ph), (e c)] fp16
        return np.ascontiguousarray(
            X.reshape(E, NPROJ, BS, PH, COLS).transpose(1, 2, 3, 0, 4)
            .reshape(NPROJ, PART, EC).astype(np.float16))

    in_maps = []
    for c in range(NCORES):
        sl = slice(c * BS, (c + 1) * BS)
        in_maps.append({
            "q": prep(Q[:, :, sl, :]),
            "k": prep(K[:, :, sl, :]),
            "v": prep(V[:, :, sl, :]),
        })

    res = run_bass_kernel_spmd(nc, in_maps, list(range(NCORES)))
    out = np.empty((B, E * P), np.float32)
    for c in range(NCORES):
        o = np.asarray(res.results[c]["out"]).astype(np.float32)
        out[c * BS:(c + 1) * BS] = (
            o.reshape(BS, PH, E, COLS).transpose(0, 2, 1, 3)
            .reshape(BS, E * P))
    return out
